# revision 10
# baseline (speedup 1.0000x reference)
"""Causal (cumulative) layer norm kernel for Trainium2, 8 NeuronCores.

Reference semantics (per (b, c) channel, running stats over time t):
    mean_t = cumsum(x)[t] / (t+1)
    var_t  = cumsum(x^2)[t] / (t+1) - mean_t^2
    out    = (x - mean_t) * rsqrt(var_t + 1e-5) * weight + bias

Sharding: data-parallel over batch B=8 -> one batch per core.

V2 (fp16 datapath):
  - x is loaded via a single SWDGE cast-DMA per wave (fp32 DRAM -> fp16
    SBUF), so no on-chip cast pass exists at all.  All big streams (x,
    x^2, scan matrices, carries, pointwise tensors) are fp16: DVE
    tensor_tensor runs in 2x mode, SBUF footprint halves, and the PE
    runs fp16 at full rate with FWL weight loads.  PSUM accumulation
    stays fp32, the tolerance budget (2e-2) is ~10x the resulting error.
  - scan matrices are pre-scaled by fp16(1/n) so the PE directly
    produces mean and E[x^2]; the SEL carry-injectors use the exact same
    fp16(1/n) values so partial-scan and carry scaling cancel exactly.
  - pointwise per pair of blocks: m2=Square(mean_ps) on ACT (PSUM src),
    var=q_ps-m2 on DVE, rstd=Rsqrt(var+eps) on ACT, xm=x-mean_ps on DVE,
    out=xm*rstd in fp32 on GPSIMD; stores ride the otherwise-idle SP
    engine (HWDGE, no cast needed since outp is fp32).
  - block 0 (t<128) needs exact math (catastrophic cancellation):
    separate fp32 load of rows 0:127 + full-fp32 matmul scans (slow PE
    path but only 2 matmuls) + the unnormalized fp32 pointwise tail
    out = (n*x - S) * rsqrt(n*Q - S^2 + eps*n^2).
"""
import numpy as np

EPS = 1e-5
B, T, C = 8, 4096, 512
P = 128                 # partitions / block size
NBLK = T // P           # 32
WAVE = 8                # blocks per wave
NWAVE = NBLK // WAVE    # 4

_CACHE = {}


def _build_consts():
    t_idx = np.arange(NBLK * P, dtype=np.float64).reshape(NBLK, P)
    inv_n16 = (1.0 / (t_idx + 1.0)).astype(np.float16)  # [blk, p]
    U = np.triu(np.ones((P, P), np.float64), k=0)

    # fp16 blob [128, W]: 31 scaled scan matrices | HOT31 | L18 | TOT2 | SEL
    uinv_w = 31 * P
    sel_w = 62 * P
    W = uinv_w + 32 + 32 + 32 + P + sel_w
    hb = np.zeros((P, W), dtype=np.float16)
    for b in range(1, NBLK):
        hb[:, (b - 1) * P:b * P] = (U * inv_n16[b][None, :].astype(np.float64)
                                    ).astype(np.float16)
    o_hot = uinv_w
    hb[:, o_hot + 15] = 1.0                      # HOT31 at [o_hot : o_hot+31]
    o_l18 = uinv_w + 32
    # L18 lhsT [K=16, M=18]: cols 0..7 = exclusive x-carries, col 8 = x total,
    # cols 9..16 = exclusive q-carries, col 17 = q total
    L18 = np.zeros((16, 18), np.float16)
    for m in range(8):
        L18[0:m, m] = 1.0
        L18[8:8 + m, 9 + m] = 1.0
    L18[0:8, 8] = 1.0
    L18[8:16, 17] = 1.0
    hb[0:16, o_l18:o_l18 + 18] = L18
    o_tot = uinv_w + 64
    hb[8, o_tot:o_tot + 9] = 1.0        # prev totx -> cols 0..8
    hb[17, o_tot + 9:o_tot + 18] = 1.0  # prev totq -> cols 9..17
    o_negi = uinv_w + 96
    hb[:, o_negi:o_negi + P] = (-np.eye(P)).astype(np.float16)
    o_sel = o_negi + P
    # selectors for block b (1..31), within-wave index i = b % 8:
    #   x-sel window 2*(b-1):   row i     = fp16(inv_n[b])
    #   q-sel window 2*(b-1)+1: row 9 + i = fp16(inv_n[b])
    for b in range(1, NBLK):
        i = b % WAVE
        w0 = o_sel + 2 * (b - 1) * P
        hb[i, w0:w0 + P] = inv_n16[b]
        hb[9 + i, w0 + P:w0 + 2 * P] = inv_n16[b]

    # fp32 blob: block 0 raw scan matrix + per-partition scalar columns
    fb = np.zeros((P, P + 3), dtype=np.float32)
    fb[:, 0:P] = U.astype(np.float32)
    fb[:, P] = (-1.0 / (t_idx[0] + 1.0)).astype(np.float32)       # -1/n
    fb[:, P + 1] = (t_idx[0] + 1.0).astype(np.float32)            # n
    fb[:, P + 2] = (EPS * (t_idx[0] + 1.0) ** 2).astype(np.float32)  # eps*n^2
    offs = {"hot": o_hot, "l18": o_l18, "tot": o_tot, "negi": o_negi,
            "sel": o_sel, "w": W}
    return fb, hb, offs


def _build_program(iters=1):
    import concourse.bacc as bacc
    import concourse.tile as tile
    from concourse import mybir

    dt = mybir.dt
    AF = mybir.ActivationFunctionType
    ALU = mybir.AluOpType

    _, _, offs = _build_consts()
    W = offs["w"]

    nc = bacc.Bacc()
    x_d = nc.declare_dram_parameter("x", [T, C], dt.float32, isOutput=False)
    fb_d = nc.declare_dram_parameter("fblob", [P, P + 3], dt.float32, isOutput=False)
    hb_d = nc.declare_dram_parameter("hblob", [P, W], dt.float16, isOutput=False)
    y_d = nc.declare_dram_parameter("y", [T, C], dt.float32, isOutput=True)

    x_v = x_d[:, :].rearrange("(n p) c -> p n c", p=P)   # [128, 32, 512] fp32
    y_v = y_d[:, :].rearrange("(n p) c -> p n c", p=P)

    def raw_rsqrt(out_ap, in_ap, bias_ap):
        eng = nc.scalar
        ins = [eng.lower_ap(in_ap), eng.lower_ap(bias_ap),
               mybir.ImmediateValue(dtype=dt.float32, value=1.0),
               mybir.ImmediateValue(dtype=dt.float32, value=0.0)]
        return eng.add_instruction(mybir.InstActivation(
            name=nc.get_next_instruction_name(), func=AF.Rsqrt,
            ins=ins, outs=[eng.lower_ap(out_ap)]))

    with tile.TileContext(nc) as tc:
        with (
            tc.tile_pool(name="consts", bufs=1) as consts,
            tc.tile_pool(name="waves", bufs=2) as waves,
            tc.tile_pool(name="small", bufs=2) as small,
            tc.tile_pool(name="blkt", bufs=4) as blkt,
            tc.tile_pool(name="blk1", bufs=1) as blk1,
            tc.tile_pool(name="ps_m", bufs=3, space="PSUM") as ps_m,
            tc.tile_pool(name="ps_q", bufs=3, space="PSUM") as ps_q,
            tc.tile_pool(name="ps_small", bufs=1, space="PSUM") as ps_small,
        ):
            fb = consts.tile([P, P + 3], dt.float32, tag="fb")
            nc.sync.dma_start(out=fb, in_=fb_d[:, :])
            hb = consts.tile([P, W], dt.float16, tag="hb")
            nc.sync.dma_start(out=hb, in_=hb_d[:, :])
            eps_t = consts.tile([P, 1], dt.float32, tag="eps")
            nc.vector.memset(eps_t, EPS)

            U0f = fb[:, 0:P]
            neginv0 = fb[:, P:P + 1]
            nvec0 = fb[:, P + 1:P + 2]
            epsn2_0 = fb[:, P + 2:P + 3]

            def Uinv(b):        # [128,128] fp16 scaled scan lhsT, b in 1..31
                return hb[:, (b - 1) * P:b * P]

            def HOT(j):         # [128,16] one-hot col j (j in 0..15)
                return hb[:, offs["hot"] + 15 - j:offs["hot"] + 31 - j]

            L18r = hb[0:16, offs["l18"]:offs["l18"] + 18]
            TOT2r = hb[0:18, offs["tot"]:offs["tot"] + 18]
            negI = hb[:, offs["negi"]:offs["negi"] + P]

            def SELx(b):        # [18,128] fp16 scaled x-carry selector
                return hb[0:18, offs["sel"] + 2 * (b - 1) * P:
                          offs["sel"] + (2 * (b - 1) + 1) * P]

            def SELq(b):
                return hb[0:18, offs["sel"] + (2 * (b - 1) + 1) * P:
                          offs["sel"] + (2 * b) * P]

            import contextlib
            loop_cm = tc.For_i(0, iters, 1) if iters > 1 else \
                contextlib.nullcontext()
            with loop_cm:
                prev_carr = None
                for w in range(NWAVE):
                    xw = waves.tile([P, WAVE, C], dt.float16, tag="xw")
                    # SWDGE cast-DMA: fp32 DRAM -> fp16 SBUF
                    nc.gpsimd.dma_start(
                        out=xw, in_=x_v[:, w * WAVE:(w + 1) * WAVE, :])
                    sq = waves.tile([P, WAVE, C], dt.float16, tag="sq")
                    outw = waves.tile([P, WAVE, C], dt.float16, tag="outw")
                    # x^2 on GPSIMD quads (keeps ACT/DVE for the block chain)
                    nc.gpsimd.tensor_tensor(out=sq[:, 0:4, :],
                                            in0=xw[:, 0:4, :],
                                            in1=xw[:, 0:4, :], op=ALU.mult)
                    nc.gpsimd.tensor_tensor(out=sq[:, 4:8, :],
                                            in0=xw[:, 4:8, :],
                                            in1=xw[:, 4:8, :], op=ALU.mult)
                    if w == 0:
                        # exact fp32 copy of block 0 for the t<128 tail
                        x0 = blk1.tile([P, C], dt.float32, tag="x0")
                        nc.gpsimd.dma_start(out=x0, in_=x_v[:, 0, :])
                        sq0 = blk1.tile([P, C], dt.float32, tag="sq0")
                        nc.scalar.square(out=sq0, in_=x0)

                    # block sums of x and x^2 -> one [16,512] psum tile
                    bs_ps = ps_small.tile([16, C], dt.float32, tag="bs")
                    for i in range(WAVE):
                        nc.tensor.matmul(bs_ps, HOT(i), xw[:, i, :],
                                         start=(i == 0), stop=False)
                    for i in range(WAVE):
                        nc.tensor.matmul(bs_ps, HOT(8 + i), sq[:, i, :],
                                         start=False, stop=(i == WAVE - 1))
                    bs_sb = small.tile([16, C], dt.float16, tag="bs_sb")
                    nc.vector.tensor_copy(out=bs_sb, in_=bs_ps)

                    # carries [18,512] = L18 @ bs (+ prev totals)
                    ca_ps = ps_small.tile([18, C], dt.float32, tag="carr")
                    first = prev_carr is None
                    nc.tensor.matmul(ca_ps, L18r, bs_sb, start=True, stop=first)
                    if not first:
                        nc.tensor.matmul(ca_ps, TOT2r, prev_carr,
                                         start=False, stop=True)
                    carr = small.tile([18, C], dt.float16, tag="carr_sb")
                    nc.vector.tensor_copy(out=carr, in_=ca_ps)
                    prev_carr = carr

                    for j in range(WAVE):
                        bidx = w * WAVE + j
                        if bidx == 0:
                            # block 0: raw S/Q via full-fp32 matmuls, exact
                            # unnormalized tail; generic path skipped.
                            s0_ps = ps_m.tile([P, C], dt.float32, tag="m")
                            q0_ps = ps_q.tile([P, C], dt.float32, tag="q")
                            nc.tensor.matmul(s0_ps, U0f, x0,
                                             start=True, stop=True)
                            nc.tensor.matmul(q0_ps, U0f, sq0,
                                             start=True, stop=True)
                            xm0 = blk1.tile([P, C], dt.float32, tag="xm0")
                            nc.vector.scalar_tensor_tensor(
                                out=xm0, in0=s0_ps, scalar=neginv0,
                                in1=x0, op0=ALU.mult, op1=ALU.add)
                            s2_0 = blk1.tile([P, C], dt.float32, tag="s2_0")
                            nc.scalar.square(out=s2_0, in_=s0_ps)
                            d0 = blk1.tile([P, C], dt.float32, tag="d0")
                            nc.vector.scalar_tensor_tensor(
                                out=d0, in0=q0_ps, scalar=nvec0, in1=s2_0,
                                op0=ALU.mult, op1=ALU.subtract)
                            r0 = blk1.tile([P, C], dt.float32, tag="r0")
                            raw_rsqrt(r0, d0, epsn2_0)
                            out0 = blk1.tile([P, C], dt.float32, tag="out0")
                            nc.vector.scalar_tensor_tensor(
                                out=out0, in0=xm0, scalar=nvec0,
                                in1=r0, op0=ALU.mult, op1=ALU.mult)
                            nc.gpsimd.dma_start(out=y_v[:, 0, :], in_=out0)
                            continue

                        mean_ps = ps_m.tile([P, C], dt.float32, tag="m")
                        q_ps = ps_q.tile([P, C], dt.float32, tag="q")
                        nc.tensor.matmul(mean_ps, Uinv(bidx), xw[:, j, :],
                                         start=True, stop=False)
                        nc.tensor.matmul(mean_ps, SELx(bidx), carr,
                                         start=False, stop=True)
                        nc.tensor.matmul(q_ps, Uinv(bidx), sq[:, j, :],
                                         start=True, stop=False)
                        nc.tensor.matmul(q_ps, SELq(bidx), carr,
                                         start=False, stop=False)
                        m2 = blkt.tile([P, C], dt.float16, tag="m2")
                        nc.scalar.square(out=m2, in_=mean_ps)
                        # -I matmul folds -mean^2 into q_ps -> it becomes var
                        nc.tensor.matmul(q_ps, negI, m2,
                                         start=False, stop=True)
                        rstd = blkt.tile([P, C], dt.float16, tag="rstd")
                        raw_rsqrt(rstd, q_ps, eps_t[:, :])
                        xm = blkt.tile([P, C], dt.float16, tag="xm")
                        nc.vector.tensor_tensor(
                            out=xm, in0=xw[:, j, :], in1=mean_ps,
                            op=ALU.subtract)
                        nc.vector.tensor_tensor(out=outw[:, j, :], in0=xm,
                                                in1=rstd, op=ALU.mult)

                    # wave-batched SWDGE store (same DGE path as the loads:
                    # mixing SWDGE loads with HWDGE stores measured ~1.5x
                    # slower on the DMA engines)
                    lo = 1 if w == 0 else 0
                    nc.gpsimd.dma_start(
                        out=y_v[:, w * WAVE + lo:(w + 1) * WAVE, :],
                        in_=outw[:, lo:WAVE, :])
    nc.compile()
    return nc


def kernel(x, weight, bias):
    from concourse.bass_utils import run_bass_kernel_spmd

    x = np.ascontiguousarray(np.asarray(x), dtype=np.float32)
    w = np.asarray(weight, dtype=np.float32).reshape(-1)
    b = np.asarray(bias, dtype=np.float32).reshape(-1)

    if "nc" not in _CACHE:
        fb, hb, _ = _build_consts()
        _CACHE["nc"] = _build_program()
        _CACHE["fb"], _CACHE["hb"] = fb, hb
    nc = _CACHE["nc"]

    in_maps = [{"x": x[core], "fblob": _CACHE["fb"], "hblob": _CACHE["hb"]}
               for core in range(B)]
    res = run_bass_kernel_spmd(nc, in_maps, list(range(B)))
    y = np.stack([res.results[core]["y"] for core in range(B)], axis=0)

    trivial = np.all(w == 1.0) and np.all(b == 0.0)
    if not trivial:
        y = y * w[None, None, :] + b[None, None, :]
    return y


# revision 13
# speedup vs baseline: 1.2374x; 1.2374x over previous
"""Causal (cumulative) layer norm kernel for Trainium2, 8 NeuronCores.

Reference semantics (per (b, c) channel, running stats over time t):
    mean_t = cumsum(x)[t] / (t+1)
    var_t  = cumsum(x^2)[t] / (t+1) - mean_t^2
    out    = (x - mean_t) * rsqrt(var_t + 1e-5) * weight + bias

Sharding: data-parallel over batch B=8 -> one batch per core.

V2 (fp16 datapath):
  - x is loaded via a single SWDGE cast-DMA per wave (fp32 DRAM -> fp16
    SBUF), so no on-chip cast pass exists at all.  All big streams (x,
    x^2, scan matrices, carries, pointwise tensors) are fp16: DVE
    tensor_tensor runs in 2x mode, SBUF footprint halves, and the PE
    runs fp16 at full rate with FWL weight loads.  PSUM accumulation
    stays fp32, the tolerance budget (2e-2) is ~10x the resulting error.
  - scan matrices are pre-scaled by fp16(1/n) so the PE directly
    produces mean and E[x^2]; the SEL carry-injectors use the exact same
    fp16(1/n) values so partial-scan and carry scaling cancel exactly.
  - pointwise per pair of blocks: m2=Square(mean_ps) on ACT (PSUM src),
    var=q_ps-m2 on DVE, rstd=Rsqrt(var+eps) on ACT, xm=x-mean_ps on DVE,
    out=xm*rstd in fp32 on GPSIMD; stores ride the otherwise-idle SP
    engine (HWDGE, no cast needed since outp is fp32).
  - block 0 (t<128) needs exact math (catastrophic cancellation):
    separate fp32 load of rows 0:127 + full-fp32 matmul scans (slow PE
    path but only 2 matmuls) + the unnormalized fp32 pointwise tail
    out = (n*x - S) * rsqrt(n*Q - S^2 + eps*n^2).
"""
import numpy as np

EPS = 1e-5
B, T, C = 8, 4096, 512
P = 128                 # partitions / block size
NBLK = T // P           # 32
WAVE = 8                # blocks per wave
NWAVE = NBLK // WAVE    # 4

_CACHE = {}
_UNROLL = 2


def _build_consts():
    t_idx = np.arange(NBLK * P, dtype=np.float64).reshape(NBLK, P)
    inv_n16 = (1.0 / (t_idx + 1.0)).astype(np.float16)  # [blk, p]
    U = np.triu(np.ones((P, P), np.float64), k=0)

    # fp16 blob [128, W]: 31 scaled scan matrices | HOT31 | L18 | TOT2 | SEL
    uinv_w = 31 * P
    sel_w = 62 * P
    W = uinv_w + 32 + 32 + 32 + P + sel_w
    hb = np.zeros((P, W), dtype=np.float16)
    for b in range(1, NBLK):
        hb[:, (b - 1) * P:b * P] = (U * inv_n16[b][None, :].astype(np.float64)
                                    ).astype(np.float16)
    o_hot = uinv_w
    hb[:, o_hot + 15] = 1.0                      # HOT31 at [o_hot : o_hot+31]
    o_l18 = uinv_w + 32
    # L18 lhsT [K=16, M=18]: cols 0..7 = exclusive x-carries, col 8 = x total,
    # cols 9..16 = exclusive q-carries, col 17 = q total
    L18 = np.zeros((16, 18), np.float16)
    for m in range(8):
        L18[0:m, m] = 1.0
        L18[8:8 + m, 9 + m] = 1.0
    L18[0:8, 8] = 1.0
    L18[8:16, 17] = 1.0
    hb[0:16, o_l18:o_l18 + 18] = L18
    o_tot = uinv_w + 64
    hb[8, o_tot:o_tot + 9] = 1.0        # prev totx -> cols 0..8
    hb[17, o_tot + 9:o_tot + 18] = 1.0  # prev totq -> cols 9..17
    o_negi = uinv_w + 96
    hb[:, o_negi:o_negi + P] = (-np.eye(P)).astype(np.float16)
    o_sel = o_negi + P
    # selectors for block b (1..31), within-wave index i = b % 8:
    #   x-sel window 2*(b-1):   row i     = fp16(inv_n[b])
    #   q-sel window 2*(b-1)+1: row 9 + i = fp16(inv_n[b])
    for b in range(1, NBLK):
        i = b % WAVE
        w0 = o_sel + 2 * (b - 1) * P
        hb[i, w0:w0 + P] = inv_n16[b]
        hb[9 + i, w0 + P:w0 + 2 * P] = inv_n16[b]

    # fp32 blob: block 0 raw scan matrix + per-partition scalar columns
    fb = np.zeros((P, P + 3), dtype=np.float32)
    fb[:, 0:P] = U.astype(np.float32)
    fb[:, P] = (-1.0 / (t_idx[0] + 1.0)).astype(np.float32)       # -1/n
    fb[:, P + 1] = (t_idx[0] + 1.0).astype(np.float32)            # n
    fb[:, P + 2] = (EPS * (t_idx[0] + 1.0) ** 2).astype(np.float32)  # eps*n^2
    offs = {"hot": o_hot, "l18": o_l18, "tot": o_tot, "negi": o_negi,
            "sel": o_sel, "w": W}
    return fb, hb, offs


def _build_program(iters=1):
    import concourse.bacc as bacc
    import concourse.tile as tile
    from concourse import mybir

    dt = mybir.dt
    AF = mybir.ActivationFunctionType
    ALU = mybir.AluOpType

    _, _, offs = _build_consts()
    W = offs["w"]

    nc = bacc.Bacc()
    x_d = nc.declare_dram_parameter("x", [T, C], dt.float32, isOutput=False)
    fb_d = nc.declare_dram_parameter("fblob", [P, P + 3], dt.float32, isOutput=False)
    hb_d = nc.declare_dram_parameter("hblob", [P, W], dt.float16, isOutput=False)
    y_d = nc.declare_dram_parameter("y", [T, C], dt.float32, isOutput=True)

    x_v = x_d[:, :].rearrange("(n p) c -> p n c", p=P)   # [128, 32, 512] fp32
    y_v = y_d[:, :].rearrange("(n p) c -> p n c", p=P)

    def raw_rsqrt(out_ap, in_ap, bias_ap):
        eng = nc.scalar
        ins = [eng.lower_ap(in_ap), eng.lower_ap(bias_ap),
               mybir.ImmediateValue(dtype=dt.float32, value=1.0),
               mybir.ImmediateValue(dtype=dt.float32, value=0.0)]
        return eng.add_instruction(mybir.InstActivation(
            name=nc.get_next_instruction_name(), func=AF.Rsqrt,
            ins=ins, outs=[eng.lower_ap(out_ap)]))

    with tile.TileContext(nc) as tc:
        with (
            tc.tile_pool(name="consts", bufs=1) as consts,
            tc.tile_pool(name="waves", bufs=3) as waves,
            tc.tile_pool(name="small", bufs=2) as small,
            tc.tile_pool(name="blk2", bufs=3) as blk2,
            tc.tile_pool(name="blk1", bufs=1) as blk1,
            tc.tile_pool(name="ps_mq", bufs=3, space="PSUM") as ps_mq,
            tc.tile_pool(name="ps_small", bufs=1, space="PSUM") as ps_small,
        ):
            fb = consts.tile([P, P + 3], dt.float32, tag="fb")
            nc.sync.dma_start(out=fb, in_=fb_d[:, :])
            hb = consts.tile([P, W], dt.float16, tag="hb")
            nc.sync.dma_start(out=hb, in_=hb_d[:, :])
            eps_t = consts.tile([P, 1], dt.float32, tag="eps")
            nc.vector.memset(eps_t, EPS)

            U0f = fb[:, 0:P]
            neginv0 = fb[:, P:P + 1]
            nvec0 = fb[:, P + 1:P + 2]
            epsn2_0 = fb[:, P + 2:P + 3]

            def Uinv(b):        # [128,128] fp16 scaled scan lhsT, b in 1..31
                return hb[:, (b - 1) * P:b * P]

            def HOT(j):         # [128,16] one-hot col j (j in 0..15)
                return hb[:, offs["hot"] + 15 - j:offs["hot"] + 31 - j]

            L18r = hb[0:16, offs["l18"]:offs["l18"] + 18]
            TOT2r = hb[0:18, offs["tot"]:offs["tot"] + 18]
            negI = hb[:, offs["negi"]:offs["negi"] + P]

            def SELx(b):        # [18,128] fp16 scaled x-carry selector
                return hb[0:18, offs["sel"] + 2 * (b - 1) * P:
                          offs["sel"] + (2 * (b - 1) + 1) * P]

            def SELq(b):
                return hb[0:18, offs["sel"] + (2 * (b - 1) + 1) * P:
                          offs["sel"] + (2 * b) * P]

            import contextlib
            # unroll _UNROLL logical iterations per hardware-loop trip so the
            # scheduler overlaps the DMA tail of one with the head of the next
            assert iters == 1 or iters % _UNROLL == 0
            loop_cm = tc.For_i(0, iters // _UNROLL, 1) if iters > 1 else \
                contextlib.nullcontext()
            with loop_cm:
              for _rep in range(1 if iters == 1 else _UNROLL):
                prev_carr = None
                for w in range(NWAVE):
                    xw = waves.tile([P, WAVE, C], dt.float16, tag="xw")
                    # SWDGE cast-DMA: fp32 DRAM -> fp16 SBUF
                    nc.gpsimd.dma_start(
                        out=xw, in_=x_v[:, w * WAVE:(w + 1) * WAVE, :])
                    sq = waves.tile([P, WAVE, C], dt.float16, tag="sq")
                    outw = waves.tile([P, WAVE, C], dt.float16, tag="outw")
                    # x^2 split across ACT / DVE quads
                    nc.scalar.square(out=sq[:, 0:4, :], in_=xw[:, 0:4, :])
                    nc.vector.tensor_tensor(out=sq[:, 4:8, :],
                                            in0=xw[:, 4:8, :],
                                            in1=xw[:, 4:8, :], op=ALU.mult)
                    if w == 0:
                        # exact fp32 copy of block 0 for the t<128 tail
                        x0 = blk1.tile([P, C], dt.float32, tag="x0")
                        nc.gpsimd.dma_start(out=x0, in_=x_v[:, 0, :])
                        sq0 = blk1.tile([P, C], dt.float32, tag="sq0")
                        nc.scalar.square(out=sq0, in_=x0)

                    # block sums of x and x^2 -> one [16,512] psum tile
                    bs_ps = ps_small.tile([16, C], dt.float32, tag="bs")
                    for i in range(WAVE):
                        nc.tensor.matmul(bs_ps, HOT(i), xw[:, i, :],
                                         start=(i == 0), stop=False)
                    for i in range(WAVE):
                        nc.tensor.matmul(bs_ps, HOT(8 + i), sq[:, i, :],
                                         start=False, stop=(i == WAVE - 1))
                    bs_sb = small.tile([16, C], dt.float16, tag="bs_sb")
                    nc.vector.tensor_copy(out=bs_sb, in_=bs_ps)

                    # carries [18,512] = L18 @ bs (+ prev totals)
                    ca_ps = ps_small.tile([18, C], dt.float32, tag="carr")
                    first = prev_carr is None
                    nc.tensor.matmul(ca_ps, L18r, bs_sb, start=True, stop=first)
                    if not first:
                        nc.tensor.matmul(ca_ps, TOT2r, prev_carr,
                                         start=False, stop=True)
                    carr = small.tile([18, C], dt.float16, tag="carr_sb")
                    nc.vector.tensor_copy(out=carr, in_=ca_ps)
                    prev_carr = carr

                    # pairs: scans + pointwise at pair granularity
                    for i2 in range(4):
                        mean_ps = ps_mq.tile([P, 2, C], dt.float32, tag="mq")
                        q_ps = ps_mq.tile([P, 2, C], dt.float32, tag="mq")
                        for h in range(2):
                            bidx = w * WAVE + 2 * i2 + h
                            if bidx == 0:
                                # raw S and Q via full-fp32 matmuls (4x
                                # stream cost but only one block)
                                nc.tensor.matmul(mean_ps[:, 0, :], U0f, x0,
                                                 start=True, stop=True)
                                nc.tensor.matmul(q_ps[:, 0, :], U0f, sq0,
                                                 start=True, stop=True)
                            else:
                                j = 2 * i2 + h
                                nc.tensor.matmul(mean_ps[:, h, :], Uinv(bidx),
                                                 xw[:, j, :],
                                                 start=True, stop=False)
                                nc.tensor.matmul(mean_ps[:, h, :], SELx(bidx),
                                                 carr, start=False, stop=True)
                                nc.tensor.matmul(q_ps[:, h, :], Uinv(bidx),
                                                 sq[:, j, :],
                                                 start=True, stop=False)
                                nc.tensor.matmul(q_ps[:, h, :], SELq(bidx),
                                                 carr, start=False, stop=False)
                        m2 = blk2.tile([P, 2, C], dt.float16, tag="m2")
                        nc.scalar.square(out=m2, in_=mean_ps)
                        # -I matmuls fold -mean^2 into q_ps -> it becomes var
                        # (block 0's half keeps raw Q >= 0: rsqrt stays sane
                        # and that half is discarded anyway)
                        for h in range(2):
                            if w == 0 and i2 == 0 and h == 0:
                                continue
                            nc.tensor.matmul(q_ps[:, h, :], negI, m2[:, h, :],
                                             start=False, stop=True)
                        rstd = blk2.tile([P, 2, C], dt.float16, tag="rstd")
                        raw_rsqrt(rstd, q_ps, eps_t[:, :])
                        xm = blk2.tile([P, 2, C], dt.float16, tag="xm")
                        nc.vector.tensor_tensor(
                            out=xm, in0=xw[:, 2 * i2:2 * i2 + 2, :],
                            in1=mean_ps, op=ALU.subtract)
                        outp = outw[:, 2 * i2:2 * i2 + 2, :]
                        nc.vector.tensor_tensor(out=outp, in0=xm, in1=rstd,
                                                op=ALU.mult)
                        if w == 0 and i2 == 0:
                            # block 0: exact path from the raw S/Q in half 0
                            s_ps0 = mean_ps[:, 0, :]
                            q_ps0 = q_ps[:, 0, :]
                            xm0 = blk1.tile([P, C], dt.float32, tag="xm0")
                            nc.vector.scalar_tensor_tensor(
                                out=xm0, in0=s_ps0, scalar=neginv0,
                                in1=x0, op0=ALU.mult, op1=ALU.add)
                            s2_0 = blk1.tile([P, C], dt.float32, tag="s2_0")
                            nc.scalar.square(out=s2_0, in_=s_ps0)
                            d0 = blk1.tile([P, C], dt.float32, tag="d0")
                            nc.vector.scalar_tensor_tensor(
                                out=d0, in0=q_ps0, scalar=nvec0, in1=s2_0,
                                op0=ALU.mult, op1=ALU.subtract)
                            r0 = blk1.tile([P, C], dt.float32, tag="r0")
                            raw_rsqrt(r0, d0, epsn2_0)
                            out0 = blk1.tile([P, C], dt.float32, tag="out0")
                            nc.vector.scalar_tensor_tensor(
                                out=out0, in0=xm0, scalar=nvec0,
                                in1=r0, op0=ALU.mult, op1=ALU.mult)
                            nc.gpsimd.dma_start(out=y_v[:, 0, :],
                                                in_=out0)
                    # wave-batched SWDGE store (same DGE path as the loads:
                    # mixing SWDGE loads with HWDGE stores measured ~1.5x
                    # slower on the DMA engines)
                    lo = 1 if w == 0 else 0
                    nc.gpsimd.dma_start(
                        out=y_v[:, w * WAVE + lo:(w + 1) * WAVE, :],
                        in_=outw[:, lo:WAVE, :])
    nc.compile()
    return nc


def kernel(x, weight, bias):
    from concourse.bass_utils import run_bass_kernel_spmd

    x = np.ascontiguousarray(np.asarray(x), dtype=np.float32)
    w = np.asarray(weight, dtype=np.float32).reshape(-1)
    b = np.asarray(bias, dtype=np.float32).reshape(-1)

    if "nc" not in _CACHE:
        fb, hb, _ = _build_consts()
        _CACHE["nc"] = _build_program()
        _CACHE["fb"], _CACHE["hb"] = fb, hb
    nc = _CACHE["nc"]

    in_maps = [{"x": x[core], "fblob": _CACHE["fb"], "hblob": _CACHE["hb"]}
               for core in range(B)]
    res = run_bass_kernel_spmd(nc, in_maps, list(range(B)))
    y = np.stack([res.results[core]["y"] for core in range(B)], axis=0)

    trivial = np.all(w == 1.0) and np.all(b == 0.0)
    if not trivial:
        y = y * w[None, None, :] + b[None, None, :]
    return y


# revision 14
# speedup vs baseline: 1.3684x; 1.1059x over previous
"""Causal (cumulative) layer norm kernel for Trainium2, 8 NeuronCores.

Reference semantics (per (b, c) channel, running stats over time t):
    mean_t = cumsum(x)[t] / (t+1)
    var_t  = cumsum(x^2)[t] / (t+1) - mean_t^2
    out    = (x - mean_t) * rsqrt(var_t + 1e-5) * weight + bias

Sharding: data-parallel over batch B=8 -> one batch per core.

V2 (fp16 datapath):
  - x is loaded via a single SWDGE cast-DMA per wave (fp32 DRAM -> fp16
    SBUF), so no on-chip cast pass exists at all.  All big streams (x,
    x^2, scan matrices, carries, pointwise tensors) are fp16: DVE
    tensor_tensor runs in 2x mode, SBUF footprint halves, and the PE
    runs fp16 at full rate with FWL weight loads.  PSUM accumulation
    stays fp32, the tolerance budget (2e-2) is ~10x the resulting error.
  - scan matrices are pre-scaled by fp16(1/n) so the PE directly
    produces mean and E[x^2]; the SEL carry-injectors use the exact same
    fp16(1/n) values so partial-scan and carry scaling cancel exactly.
  - pointwise per pair of blocks: m2=Square(mean_ps) on ACT (PSUM src),
    var=q_ps-m2 on DVE, rstd=Rsqrt(var+eps) on ACT, xm=x-mean_ps on DVE,
    out=xm*rstd in fp32 on GPSIMD; stores ride the otherwise-idle SP
    engine (HWDGE, no cast needed since outp is fp32).
  - block 0 (t<128) needs exact math (catastrophic cancellation):
    separate fp32 load of rows 0:127 + full-fp32 matmul scans (slow PE
    path but only 2 matmuls) + the unnormalized fp32 pointwise tail
    out = (n*x - S) * rsqrt(n*Q - S^2 + eps*n^2).
"""
import numpy as np

EPS = 1e-5
B, T, C = 8, 4096, 512
P = 128                 # partitions / block size
NBLK = T // P           # 32
WAVE = 8                # blocks per wave
NWAVE = NBLK // WAVE    # 4

_CACHE = {}
_UNROLL = 4


def _build_consts():
    t_idx = np.arange(NBLK * P, dtype=np.float64).reshape(NBLK, P)
    inv_n16 = (1.0 / (t_idx + 1.0)).astype(np.float16)  # [blk, p]
    U = np.triu(np.ones((P, P), np.float64), k=0)

    # fp16 blob [128, W]: 31 scaled scan matrices | HOT31 | L18 | TOT2 | SEL
    uinv_w = 31 * P
    sel_w = 62 * P
    W = uinv_w + 32 + 32 + 32 + P + sel_w
    hb = np.zeros((P, W), dtype=np.float16)
    for b in range(1, NBLK):
        hb[:, (b - 1) * P:b * P] = (U * inv_n16[b][None, :].astype(np.float64)
                                    ).astype(np.float16)
    o_hot = uinv_w
    hb[:, o_hot + 15] = 1.0                      # HOT31 at [o_hot : o_hot+31]
    o_l18 = uinv_w + 32
    # L18 lhsT [K=16, M=18]: cols 0..7 = exclusive x-carries, col 8 = x total,
    # cols 9..16 = exclusive q-carries, col 17 = q total
    L18 = np.zeros((16, 18), np.float16)
    for m in range(8):
        L18[0:m, m] = 1.0
        L18[8:8 + m, 9 + m] = 1.0
    L18[0:8, 8] = 1.0
    L18[8:16, 17] = 1.0
    hb[0:16, o_l18:o_l18 + 18] = L18
    o_tot = uinv_w + 64
    hb[8, o_tot:o_tot + 9] = 1.0        # prev totx -> cols 0..8
    hb[17, o_tot + 9:o_tot + 18] = 1.0  # prev totq -> cols 9..17
    o_negi = uinv_w + 96
    hb[:, o_negi:o_negi + P] = (-np.eye(P)).astype(np.float16)
    o_sel = o_negi + P
    # selectors for block b (1..31), within-wave index i = b % 8:
    #   x-sel window 2*(b-1):   row i     = fp16(inv_n[b])
    #   q-sel window 2*(b-1)+1: row 9 + i = fp16(inv_n[b])
    for b in range(1, NBLK):
        i = b % WAVE
        w0 = o_sel + 2 * (b - 1) * P
        hb[i, w0:w0 + P] = inv_n16[b]
        hb[9 + i, w0 + P:w0 + 2 * P] = inv_n16[b]

    # fp32 blob: block 0 raw scan matrix + per-partition scalar columns
    fb = np.zeros((P, P + 3), dtype=np.float32)
    fb[:, 0:P] = U.astype(np.float32)
    fb[:, P] = (-1.0 / (t_idx[0] + 1.0)).astype(np.float32)       # -1/n
    fb[:, P + 1] = (t_idx[0] + 1.0).astype(np.float32)            # n
    fb[:, P + 2] = (EPS * (t_idx[0] + 1.0) ** 2).astype(np.float32)  # eps*n^2
    offs = {"hot": o_hot, "l18": o_l18, "tot": o_tot, "negi": o_negi,
            "sel": o_sel, "w": W}
    return fb, hb, offs


def _build_program(iters=1):
    import concourse.bacc as bacc
    import concourse.tile as tile
    from concourse import mybir

    dt = mybir.dt
    AF = mybir.ActivationFunctionType
    ALU = mybir.AluOpType

    _, _, offs = _build_consts()
    W = offs["w"]

    nc = bacc.Bacc()
    x_d = nc.declare_dram_parameter("x", [T, C], dt.float32, isOutput=False)
    fb_d = nc.declare_dram_parameter("fblob", [P, P + 3], dt.float32, isOutput=False)
    hb_d = nc.declare_dram_parameter("hblob", [P, W], dt.float16, isOutput=False)
    y_d = nc.declare_dram_parameter("y", [T, C], dt.float32, isOutput=True)

    x_v = x_d[:, :].rearrange("(n p) c -> p n c", p=P)   # [128, 32, 512] fp32
    y_v = y_d[:, :].rearrange("(n p) c -> p n c", p=P)

    def raw_rsqrt(out_ap, in_ap, bias_ap):
        eng = nc.scalar
        ins = [eng.lower_ap(in_ap), eng.lower_ap(bias_ap),
               mybir.ImmediateValue(dtype=dt.float32, value=1.0),
               mybir.ImmediateValue(dtype=dt.float32, value=0.0)]
        return eng.add_instruction(mybir.InstActivation(
            name=nc.get_next_instruction_name(), func=AF.Rsqrt,
            ins=ins, outs=[eng.lower_ap(out_ap)]))

    with tile.TileContext(nc) as tc:
        with (
            tc.tile_pool(name="consts", bufs=1) as consts,
            tc.tile_pool(name="waves", bufs=3) as waves,
            tc.tile_pool(name="small", bufs=2) as small,
            tc.tile_pool(name="blk2", bufs=3) as blk2,
            tc.tile_pool(name="blk1", bufs=1) as blk1,
            tc.tile_pool(name="ps_mq", bufs=3, space="PSUM") as ps_mq,
            tc.tile_pool(name="ps_small", bufs=1, space="PSUM") as ps_small,
        ):
            fb = consts.tile([P, P + 3], dt.float32, tag="fb")
            nc.sync.dma_start(out=fb, in_=fb_d[:, :])
            hb = consts.tile([P, W], dt.float16, tag="hb")
            nc.sync.dma_start(out=hb, in_=hb_d[:, :])
            eps_t = consts.tile([P, 1], dt.float32, tag="eps")
            nc.vector.memset(eps_t, EPS)

            U0f = fb[:, 0:P]
            neginv0 = fb[:, P:P + 1]
            nvec0 = fb[:, P + 1:P + 2]
            epsn2_0 = fb[:, P + 2:P + 3]

            def Uinv(b):        # [128,128] fp16 scaled scan lhsT, b in 1..31
                return hb[:, (b - 1) * P:b * P]

            def HOT(j):         # [128,16] one-hot col j (j in 0..15)
                return hb[:, offs["hot"] + 15 - j:offs["hot"] + 31 - j]

            L18r = hb[0:16, offs["l18"]:offs["l18"] + 18]
            TOT2r = hb[0:18, offs["tot"]:offs["tot"] + 18]
            negI = hb[:, offs["negi"]:offs["negi"] + P]

            def SELx(b):        # [18,128] fp16 scaled x-carry selector
                return hb[0:18, offs["sel"] + 2 * (b - 1) * P:
                          offs["sel"] + (2 * (b - 1) + 1) * P]

            def SELq(b):
                return hb[0:18, offs["sel"] + (2 * (b - 1) + 1) * P:
                          offs["sel"] + (2 * b) * P]

            import contextlib
            # unroll _UNROLL logical iterations per hardware-loop trip so the
            # scheduler overlaps the DMA tail of one with the head of the next
            assert iters == 1 or iters % _UNROLL == 0
            loop_cm = tc.For_i(0, iters // _UNROLL, 1) if iters > 1 else \
                contextlib.nullcontext()
            with loop_cm:
              for _rep in range(1 if iters == 1 else _UNROLL):
                prev_carr = None
                for w in range(NWAVE):
                    xw = waves.tile([P, WAVE, C], dt.float16, tag="xw")
                    # SWDGE cast-DMA: fp32 DRAM -> fp16 SBUF
                    nc.gpsimd.dma_start(
                        out=xw, in_=x_v[:, w * WAVE:(w + 1) * WAVE, :])
                    sq = waves.tile([P, WAVE, C], dt.float16, tag="sq")
                    outw = waves.tile([P, WAVE, C], dt.float16, tag="outw")
                    # x^2 split across ACT / DVE quads
                    nc.scalar.square(out=sq[:, 0:4, :], in_=xw[:, 0:4, :])
                    nc.vector.tensor_tensor(out=sq[:, 4:8, :],
                                            in0=xw[:, 4:8, :],
                                            in1=xw[:, 4:8, :], op=ALU.mult)
                    if w == 0:
                        # exact fp32 copy of block 0 for the t<128 tail
                        x0 = blk1.tile([P, C], dt.float32, tag="x0")
                        nc.gpsimd.dma_start(out=x0, in_=x_v[:, 0, :])
                        sq0 = blk1.tile([P, C], dt.float32, tag="sq0")
                        nc.scalar.square(out=sq0, in_=x0)

                    # block sums of x and x^2 -> one [16,512] psum tile
                    bs_ps = ps_small.tile([16, C], dt.float32, tag="bs")
                    for i in range(WAVE):
                        nc.tensor.matmul(bs_ps, HOT(i), xw[:, i, :],
                                         start=(i == 0), stop=False)
                    for i in range(WAVE):
                        nc.tensor.matmul(bs_ps, HOT(8 + i), sq[:, i, :],
                                         start=False, stop=(i == WAVE - 1))
                    bs_sb = small.tile([16, C], dt.float16, tag="bs_sb")
                    nc.vector.tensor_copy(out=bs_sb, in_=bs_ps)

                    # carries [18,512] = L18 @ bs (+ prev totals)
                    ca_ps = ps_small.tile([18, C], dt.float32, tag="carr")
                    first = prev_carr is None
                    nc.tensor.matmul(ca_ps, L18r, bs_sb, start=True, stop=first)
                    if not first:
                        nc.tensor.matmul(ca_ps, TOT2r, prev_carr,
                                         start=False, stop=True)
                    carr = small.tile([18, C], dt.float16, tag="carr_sb")
                    nc.vector.tensor_copy(out=carr, in_=ca_ps)
                    prev_carr = carr

                    # pairs: scans + pointwise at pair granularity
                    for i2 in range(4):
                        mean_ps = ps_mq.tile([P, 2, C], dt.float32, tag="mq")
                        q_ps = ps_mq.tile([P, 2, C], dt.float32, tag="mq")
                        for h in range(2):
                            bidx = w * WAVE + 2 * i2 + h
                            if bidx == 0:
                                # raw S and Q via full-fp32 matmuls (4x
                                # stream cost but only one block)
                                nc.tensor.matmul(mean_ps[:, 0, :], U0f, x0,
                                                 start=True, stop=True)
                                nc.tensor.matmul(q_ps[:, 0, :], U0f, sq0,
                                                 start=True, stop=True)
                            else:
                                j = 2 * i2 + h
                                nc.tensor.matmul(mean_ps[:, h, :], Uinv(bidx),
                                                 xw[:, j, :],
                                                 start=True, stop=False)
                                nc.tensor.matmul(mean_ps[:, h, :], SELx(bidx),
                                                 carr, start=False, stop=True)
                                nc.tensor.matmul(q_ps[:, h, :], Uinv(bidx),
                                                 sq[:, j, :],
                                                 start=True, stop=False)
                                nc.tensor.matmul(q_ps[:, h, :], SELq(bidx),
                                                 carr, start=False, stop=False)
                        m2 = blk2.tile([P, 2, C], dt.float16, tag="m2")
                        nc.scalar.square(out=m2, in_=mean_ps)
                        # -I matmuls fold -mean^2 into q_ps -> it becomes var
                        # (block 0's half keeps raw Q >= 0: rsqrt stays sane
                        # and that half is discarded anyway)
                        for h in range(2):
                            if w == 0 and i2 == 0 and h == 0:
                                continue
                            nc.tensor.matmul(q_ps[:, h, :], negI, m2[:, h, :],
                                             start=False, stop=True)
                        rstd = blk2.tile([P, 2, C], dt.float16, tag="rstd")
                        raw_rsqrt(rstd, q_ps, eps_t[:, :])
                        xm = blk2.tile([P, 2, C], dt.float16, tag="xm")
                        nc.vector.tensor_tensor(
                            out=xm, in0=xw[:, 2 * i2:2 * i2 + 2, :],
                            in1=mean_ps, op=ALU.subtract)
                        outp = outw[:, 2 * i2:2 * i2 + 2, :]
                        nc.vector.tensor_tensor(out=outp, in0=xm, in1=rstd,
                                                op=ALU.mult)
                        if w == 0 and i2 == 0:
                            # block 0: exact path from the raw S/Q in half 0
                            s_ps0 = mean_ps[:, 0, :]
                            q_ps0 = q_ps[:, 0, :]
                            xm0 = blk1.tile([P, C], dt.float32, tag="xm0")
                            nc.vector.scalar_tensor_tensor(
                                out=xm0, in0=s_ps0, scalar=neginv0,
                                in1=x0, op0=ALU.mult, op1=ALU.add)
                            s2_0 = blk1.tile([P, C], dt.float32, tag="s2_0")
                            nc.scalar.square(out=s2_0, in_=s_ps0)
                            d0 = blk1.tile([P, C], dt.float32, tag="d0")
                            nc.vector.scalar_tensor_tensor(
                                out=d0, in0=q_ps0, scalar=nvec0, in1=s2_0,
                                op0=ALU.mult, op1=ALU.subtract)
                            r0 = blk1.tile([P, C], dt.float32, tag="r0")
                            raw_rsqrt(r0, d0, epsn2_0)
                            out0 = blk1.tile([P, C], dt.float32, tag="out0")
                            nc.vector.scalar_tensor_tensor(
                                out=out0, in0=xm0, scalar=nvec0,
                                in1=r0, op0=ALU.mult, op1=ALU.mult)
                            nc.gpsimd.dma_start(out=y_v[:, 0, :],
                                                in_=out0)
                    # wave-batched SWDGE store (same DGE path as the loads:
                    # mixing SWDGE loads with HWDGE stores measured ~1.5x
                    # slower on the DMA engines)
                    lo = 1 if w == 0 else 0
                    nc.gpsimd.dma_start(
                        out=y_v[:, w * WAVE + lo:(w + 1) * WAVE, :],
                        in_=outw[:, lo:WAVE, :])
    nc.compile()
    return nc


def kernel(x, weight, bias):
    from concourse.bass_utils import run_bass_kernel_spmd

    x = np.ascontiguousarray(np.asarray(x), dtype=np.float32)
    w = np.asarray(weight, dtype=np.float32).reshape(-1)
    b = np.asarray(bias, dtype=np.float32).reshape(-1)

    if "nc" not in _CACHE:
        fb, hb, _ = _build_consts()
        _CACHE["nc"] = _build_program()
        _CACHE["fb"], _CACHE["hb"] = fb, hb
    nc = _CACHE["nc"]

    in_maps = [{"x": x[core], "fblob": _CACHE["fb"], "hblob": _CACHE["hb"]}
               for core in range(B)]
    res = run_bass_kernel_spmd(nc, in_maps, list(range(B)))
    y = np.stack([res.results[core]["y"] for core in range(B)], axis=0)

    trivial = np.all(w == 1.0) and np.all(b == 0.0)
    if not trivial:
        y = y * w[None, None, :] + b[None, None, :]
    return y
